# revision 1
# baseline (speedup 1.0000x reference)
"""Trainium2 Bass kernel for nn_Decode (3-step Time-LSTM decoder + dense stack).

Sharding: pure data parallel over batch across 8 NeuronCores (4096 rows each),
weights replicated. Device layout is feature-major (transposed): activations
are [feature_partition, batch_free] tiles, so all weights stay PE-stationary
and batch streams as the matmul moving operand (N=512 columns per chunk = one
PSUM bank at fp32).

Host-side prep (sharding/layout only):
  - slice context_state[:, 2, :] (the model reads only step 2)
  - fold the per-step attention vectors into Wx/Wxt:
        (h*aw_t) @ W == h @ (aw_t[:,None]*W)
  - transpose h to [HID, B] so the device reads feature-major data

All matmuls run as float32r (TF32-like: 1 col/cycle at N>=256; measured
~1.4e-4 rel err/matmul vs 2.2e-3 for bf16). Per step t (banks i,f,o | Tg,g):
  gates = Wk_t.T @ h_last (+ Uh.T @ h_t) (+ Wto.T t into o-bank)
          (+ sigma(Wtt.T t) accumulated into the Tg bank via identity matmul;
           sigma(Wtt_j*t_b) itself is a GpSimd partition_broadcast of t plus
           one ACT sigmoid with a per-partition scale - no PE/PSUM needed)
  c' = f*c + i*Tg*tanh(g);  h' = o*tanh(c');  out_t = relu-dense x3 (h')

Schedule notes (what made it fast - 259us -> 120us on the cost model):
  - STEP-MAJOR loop (for t: for chunk:): consecutive uses of the PSUM gate
    banks belong to different, independent batch chunks, so each chunk's
    h-recurrence latency hides behind the other chunks' gate matmuls.
  - PSUM split [i|f|o] + [Tg|g]: the 3-bank fused sigmoid is not gated by
    the S-chain, and banks recycle in two independent short cycles.
  - ACT is the binding in-order stream (~95us busy of ~120us): sigma(i,f,o)
    is emitted first per instance, the S-chain after it; relus run on DVE;
    p1/p3 products run on GpSimd to unload DVE.
  - t tiles are prefetched two instances ahead on the Pool DMA queue; bulk
    h loads are split per chunk on the sync queue in need-by order.

Fast path requires all-zero biases (true for this problem); a generic path
applies biases through ACT's per-partition bias operand.
"""
import sys

sys.path.insert(0, "/opt/trn_rl_repo")

import numpy as np
import concourse.bacc as bacc
import concourse.tile as tile
from concourse import mybir
from concourse.bass_utils import run_bass_kernel_spmd

N_CORES = 8
B = 32768
HID = 256
FEAT = 128
R = B // N_CORES        # batch rows per core
NB = 512                # batch columns per chunk (= one PSUM bank at fp32)
NCHUNK = R // NB
F32R = mybir.dt.float32r
F32 = mybir.dt.float32
BF16 = mybir.dt.bfloat16
AF = mybir.ActivationFunctionType

DEFAULT_CFG = dict(
    zero_bias=True,   # fused sigmoid across gate banks (requires zero biases)
    relu_act=0,       # of each 6 dense relus, how many run on ACT (rest on DVE)
    gate_dt="f32",    # dtype of gate/elementwise tiles: "f32" | "bf16"
    act_bufs=2,       # default buffering of the activation tile pool
    ifoT_bufs=3,      # buffering of the gate-output tiles
    fine_psum=False,  # gate banks as [i|f],[o],[Tg|g] tiles instead of [i|f|o],[Tg|g]
    pool_tt="13",    # which cell-update products run on GpSimd: subset of "13h"
)


def build_nc(cfg=None):
    cfg = {**DEFAULT_CFG, **(cfg or {})}
    zero_bias = cfg["zero_bias"]
    relu_act = cfg["relu_act"]
    act_bufs = cfg["act_bufs"]
    ifoT_bufs = cfg["ifoT_bufs"]
    fine_psum = cfg["fine_psum"]
    pool_tt = cfg["pool_tt"]
    delay_tail = cfg.get("delay_tail", False)
    merge4b = cfg.get("merge4b", False)
    GDT = F32 if cfg["gate_dt"] == "f32" else BF16

    nc = bacc.Bacc(target_bir_lowering=False)

    h_d = nc.dram_tensor("h", [2, 128, R], F32R, kind="ExternalInput")
    wk_d = nc.dram_tensor("wk", [2, 128, 3, 640], F32R, kind="ExternalInput")
    uh_d = nc.dram_tensor("uh", [128, 4, 128], F32R, kind="ExternalInput")
    dw_d = nc.dram_tensor("dw", [128, 3, 128], F32R, kind="ExternalInput")
    trow_d = nc.dram_tensor("trow", [1, 2, 128], F32R, kind="ExternalInput")
    ident_d = nc.dram_tensor("ident", [128, 128], F32R, kind="ExternalInput")
    bias_d = nc.dram_tensor("bias", [128, 9], F32, kind="ExternalInput")
    t_d = nc.dram_tensor("t", [1, 3, R], F32R, kind="ExternalInput")
    out_d = nc.dram_tensor("out", [3, 128, R], F32R, kind="ExternalOutput")

    with tile.TileContext(nc) as tc:
        with (
            tc.tile_pool(name="const", bufs=1) as const,
            tc.tile_pool(name="act", bufs=act_bufs) as act,
            tc.tile_pool(name="ps", bufs=1, space="PSUM") as ps,
        ):
            # Pool (SWDGE) queue carries only trow + the per-instance t tiles,
            # so the first instance's Wto/S chain is ready almost immediately
            trow_sb = const.tile([1, 2, 128], F32R)
            nc.gpsimd.dma_start(out=trow_sb[:], in_=trow_d[:])
            # warm the ACT table set (sigmoid/tanh/relu) before data arrives
            warm = const.tile([1, 1], F32)
            nc.vector.memset(warm[:], 0.0)
            nc.scalar.activation(warm[:], warm[:], AF.Sigmoid)
            wk_sb = const.tile([128, 2, 3, 640], F32R)
            hsb = const.tile([128, 2, R], F32R)
            ident_sb = const.tile([128, 128], F32R)
            bias_sb = const.tile([128, 9], F32)
            dw_sb = const.tile([128, 3, 128], F32R)
            uh_sb = const.tile([128, 4, 128], F32R)
            wk_r = wk_d.rearrange("a p t m -> p a t m")
            h_r = h_d.rearrange("a p n -> p a n")
            # sync queue in need-by order: wk[t0] (finely split) + h0 first,
            # then ident/dense weights, remaining h chunks, later-step weights
            for m in range(5):
                nc.sync.dma_start(out=wk_sb[:, :, 0, m * 128:(m + 1) * 128],
                                  in_=wk_r[:, :, 0, m * 128:(m + 1) * 128])
                if m == 0:
                    nc.sync.dma_start(out=hsb[:, :, 0:NB], in_=h_r[:, :, 0:NB])
            nc.sync.dma_start(out=ident_sb[:], in_=ident_d[:])
            nc.sync.dma_start(out=dw_sb[:], in_=dw_d[:])
            for c in range(1, NCHUNK):
                col = slice(c * NB, (c + 1) * NB)
                nc.sync.dma_start(out=hsb[:, :, col], in_=h_r[:, :, col])
                if c == 1:
                    nc.sync.dma_start(out=uh_sb[:], in_=uh_d[:])
                    nc.sync.dma_start(out=bias_sb[:], in_=bias_d[:])
                if c == 2:
                    nc.sync.dma_start(out=wk_sb[:, :, 1, :], in_=wk_r[:, :, 1, :])
                if c == 4:
                    nc.sync.dma_start(out=wk_sb[:, :, 2, :], in_=wk_r[:, :, 2, :])

            # recurrent state, updated in place (the write of step t happens
            # after all step-t readers of the same column range)
            h_st = const.tile([128, R], F32R, name="hst")
            c_st = const.tile([128, R], GDT, name="cst")

            t_tiles = {}

            def load_t(t, inst):
                col, nb, key = inst
                tt = act.tile([1, nb], F32R, tag="t_tile", bufs=4,
                              name=f"tt_{key}_{t}")
                nc.gpsimd.dma_start(out=tt[:], in_=t_d[:, t, col])
                t_tiles[(t, key)] = tt

            def emit_gates(t, inst, h_prev):
                """PE gate matmuls for one chunk -> (gsig, tgg, t_tile)."""
                col, nb, key = inst
                c = key
                t_tile = t_tiles.pop((t, key))
                if merge4b:
                    gsig = ps.tile([128, 4, nb], F32, tag="gsig", name=f"gsig_{c}_{t}")
                    gg = ps.tile([128, nb], F32, tag="gg", name=f"gg_{c}_{t}")
                    tgg = (gsig, gg)
                    targets = [gsig[:, 0, :], gsig[:, 1, :], gsig[:, 2, :],
                               gsig[:, 3, :], gg[:]]
                elif fine_psum:
                    if_ps = ps.tile([128, 2, nb], F32, tag="ifp", name=f"ifp_{c}_{t}")
                    o_ps = ps.tile([128, nb], F32, tag="op", name=f"op_{c}_{t}")
                    tgg = ps.tile([128, 2, nb], F32, tag="tgg", name=f"tgg_{c}_{t}")
                    gsig = (if_ps, o_ps)
                    targets = [if_ps[:, 0, :], if_ps[:, 1, :], o_ps[:],
                               tgg[:, 0, :], tgg[:, 1, :]]
                else:
                    gsig = ps.tile([128, 3, nb], F32, tag="gsig", name=f"gsig_{c}_{t}")
                    tgg = ps.tile([128, 2, nb], F32, tag="tgg", name=f"tgg_{c}_{t}")
                    targets = [gsig[:, 0, :], gsig[:, 1, :], gsig[:, 2, :],
                               tgg[:, 0, :], tgg[:, 1, :]]
                for m in range(5):
                    tgt = targets[m]
                    n_extra = (1 if m == 2 else 0) \
                        + (1 if (t > 0 and m != 3) else 0) \
                        + (1 if m == 3 else 0)
                    for k in range(2):
                        nc.tensor.matmul(
                            tgt,
                            wk_sb[:, k, t, m * 128:(m + 1) * 128],
                            hsb[:, k, col],
                            start=(k == 0),
                            stop=(k == 1 and n_extra == 0),
                        )
                    if m == 2:  # o += Wto.T t
                        n_extra -= 1
                        nc.tensor.matmul(
                            tgt, trow_sb[:, 1, :], t_tile[:],
                            start=False, stop=(n_extra == 0),
                        )
                    if t > 0 and m != 3:  # += Uh.T h_prev
                        n_extra -= 1
                        nc.tensor.matmul(
                            tgt, uh_sb[:, min(m, 3), :], h_prev[:, col],
                            start=False, stop=(n_extra == 0),
                        )
                return gsig, tgg, t_tile

            def emit_schain(t, c, nb, tgg, t_tile):
                """sigma(Wtt.T t) via GpSimd broadcast + per-partition ACT scale,
                accumulated into the Tg bank with an identity matmul."""
                tb = act.tile([128, nb], F32R, tag="tb", bufs=2, name=f"tb_{c}_{t}")
                nc.gpsimd.partition_broadcast(tb[:], t_tile[:])
                s_sb = act.tile([128, nb], F32R, tag="s_sb", bufs=2, name=f"s_{c}_{t}")
                nc.scalar.activation(s_sb[:], tb[:], AF.Sigmoid,
                                     scale=bias_sb[:, 8:9])
                tg_bank = tgg[0][:, 3, :] if merge4b else tgg[:, 0, :]
                nc.tensor.matmul(tg_bank, ident_sb[:], s_sb[:],
                                 start=False, stop=True)

            def emit_sigmas(t, c, nb, gsig, tgg, t_tile, ifoT_dst, g_dst):
                """PSUM gate banks -> sigmoid/tanh -> SBUF slices."""
                if merge4b and zero_bias:
                    gsig4, gg = tgg
                    emit_schain(t, c, nb, tgg, t_tile)
                    nc.scalar.activation(ifoT_dst[:], gsig4[:], AF.Sigmoid)
                    nc.scalar.activation(g_dst, gg[:], AF.Tanh)
                elif fine_psum and zero_bias:
                    if_ps, o_ps = gsig
                    nc.scalar.activation(ifoT_dst[:, 0:2, :], if_ps[:], AF.Sigmoid)
                    nc.scalar.activation(ifoT_dst[:, 2, :], o_ps[:], AF.Sigmoid)
                    emit_schain(t, c, nb, tgg, t_tile)
                    nc.scalar.activation(ifoT_dst[:, 3, :], tgg[:, 0, :], AF.Sigmoid)
                    nc.scalar.activation(g_dst, tgg[:, 1, :], AF.Tanh)
                elif zero_bias:
                    nc.scalar.activation(ifoT_dst[:, 0:3, :], gsig[:], AF.Sigmoid)
                    emit_schain(t, c, nb, tgg, t_tile)
                    nc.scalar.activation(ifoT_dst[:, 3, :], tgg[:, 0, :], AF.Sigmoid)
                    nc.scalar.activation(g_dst, tgg[:, 1, :], AF.Tanh)
                else:
                    if fine_psum:
                        if_ps, o_ps = gsig
                        srcs = [if_ps[:, 0, :], if_ps[:, 1, :], o_ps[:], tgg[:, 0, :]]
                    else:
                        srcs = [gsig[:, 0, :], gsig[:, 1, :], gsig[:, 2, :], tgg[:, 0, :]]
                    emit_schain(t, c, nb, tgg, t_tile)
                    for m in range(4):
                        nc.scalar.activation(ifoT_dst[:, m, :], srcs[m], AF.Sigmoid,
                                             bias=bias_sb[:, m:m + 1])
                    nc.scalar.activation(g_dst, tgg[:, 1, :], AF.Tanh,
                                         bias=bias_sb[:, 4:5])

            def emit_dense(t, inst, h_cur):
                col, nb, c = inst
                ci = int(c.rstrip("ab"))
                cur = None
                for l in range(3):
                    dps = ps.tile([128, nb], F32, tag="dps", bufs=3,
                                  name=f"dps_{c}_{t}_{l}")
                    nc.tensor.matmul(
                        dps[:], dw_sb[:, l, :],
                        h_cur[:, col] if l == 0 else cur[:],
                        start=True, stop=True,
                    )
                    dsb = act.tile([128, nb], F32R, tag=f"dsb{l}", bufs=3, name=f"d_{c}_{t}_{l}")
                    if not zero_bias:
                        nc.scalar.activation(
                            dsb[:], dps[:], AF.Relu, bias=bias_sb[:, 5 + l:6 + l]
                        )
                    elif (ci * 3 + t * 5 + l) % 6 < relu_act:
                        nc.scalar.activation(dsb[:], dps[:], AF.Relu)
                    else:
                        nc.vector.tensor_relu(dsb[:], dps[:])
                    cur = dsb
                nc.sync.dma_start(out=out_d[t, :, col], in_=cur[:])

            # t tiles are prefetched two instances ahead (inside the tail) so
            # the Pool queue never blocks the current instance's S chain.
            # The very last instance is split in half to shorten the serial
            # drain at the end of the kernel.
            def make_insts(t):
                full = [(slice(c * NB, (c + 1) * NB), NB, f"{c}") for c in range(NCHUNK)]
                if t == 2 and cfg.get("split_last", False):
                    c = NCHUNK - 1
                    h0 = slice(c * NB, c * NB + NB // 2)
                    h1 = slice(c * NB + NB // 2, (c + 1) * NB)
                    return full[:-1] + [(h0, NB // 2, f"{c}a"), (h1, NB // 2, f"{c}b")]
                return full

            inst_order = [(tt_, inst) for tt_ in range(3) for inst in make_insts(tt_)]
            load_t(*inst_order[0])
            load_t(*inst_order[1])
            gflat = [0]

            for t in range(3):
                h_prev = h_cur = h_st
                c_prev = c_cur = c_st
                for inst in make_insts(t):
                    col, nb, key = inst
                    gsig, tgg, t_tile = emit_gates(t, inst, h_prev)
                    ifoT = act.tile([128, 4, nb], GDT, tag="ifoT", bufs=ifoT_bufs,
                                    name=f"ifoT_{key}_{t}")
                    g2 = act.tile([128, nb], GDT, tag="g2", name=f"g2_{key}_{t}")
                    emit_sigmas(t, key, nb, gsig, tgg, t_tile, ifoT[:], g2[:])

                    # ---- DVE: cell update ----
                    p1 = act.tile([128, nb], GDT, tag="p1", name=f"p1_{key}_{t}")
                    eng1 = nc.gpsimd if "1" in pool_tt else nc.vector
                    eng1.tensor_mul(p1[:], ifoT[:, 3, :], g2[:])
                    if t == 0:
                        nc.vector.tensor_mul(c_cur[:, col], ifoT[:, 0, :], p1[:])
                    else:
                        p2 = act.tile([128, nb], GDT, tag="p2", name=f"p2_{key}_{t}")
                        nc.vector.tensor_mul(p2[:], ifoT[:, 0, :], p1[:])
                        p3 = act.tile([128, nb], GDT, tag="p3", name=f"p3_{key}_{t}")
                        eng3 = nc.gpsimd if "3" in pool_tt else nc.vector
                        eng3.tensor_mul(p3[:], ifoT[:, 1, :], c_prev[:, col])
                        nc.vector.tensor_add(c_cur[:, col], p2[:], p3[:])
                    tanh_c = act.tile([128, nb], GDT, tag="tanh_c", name=f"tc_{key}_{t}")
                    nc.scalar.activation(tanh_c[:], c_cur[:, col], AF.Tanh)
                    engh = nc.gpsimd if "h" in pool_tt else nc.vector
                    engh.tensor_mul(h_cur[:, col], ifoT[:, 2, :], tanh_c[:])

                    gflat_now = gflat[0]
                    gflat[0] += 1
                    if gflat_now + 2 < len(inst_order):
                        load_t(*inst_order[gflat_now + 2])
                    emit_dense(t, inst, h_cur)

    nc.finalize()
    return nc


_NC_CACHE = {}


def _get_nc(key, cfg):
    if key not in _NC_CACHE:
        _NC_CACHE[key] = build_nc(cfg)
    return _NC_CACHE[key]


def kernel(context_state, input_t, aw1, aw2, aw3, Wx, Uh, b,
           Wxt, Wtt, bt, Wto, w1, b1, w2, b2, w3, b3):
    f32 = np.float32
    f64 = np.float64

    # ---- host-side prep / sharding ----
    h_last = np.asarray(context_state)[:, 2, :].astype(f32)          # [B, HID]
    hT = np.ascontiguousarray(h_last.T).reshape(2, 128, B)           # [2,128,B]
    tT = np.ascontiguousarray(np.asarray(input_t)[:, 3:, 0].T)       # [3, B]
    aw = np.concatenate(
        [np.asarray(aw1), np.asarray(aw2), np.asarray(aw3)], axis=1
    )[0].astype(f64)                                                 # [3, HID]

    Wx64, Wxt64 = np.asarray(Wx, f64), np.asarray(Wxt, f64)
    wk = np.empty((HID, 3, 640), f64)
    for t in range(3):
        wxf = aw[t][:, None] * Wx64                                  # [HID, 512]
        wtf = aw[t][:, None] * Wxt64                                 # [HID, 128]
        wk[:, t, 0:384] = wxf[:, 0:384]      # i, f, o
        wk[:, t, 384:512] = wtf              # Tg
        wk[:, t, 512:640] = wxf[:, 384:512]  # g
    wk = np.ascontiguousarray(wk.astype(f32)).reshape(2, 128, 3, 640)

    uh = np.ascontiguousarray(np.asarray(Uh, f32).reshape(128, 4, 128))
    dw = np.ascontiguousarray(np.stack(
        [np.asarray(w1, f32), np.asarray(w2, f32), np.asarray(w3, f32)], axis=1))
    trow = np.ascontiguousarray(
        np.stack([np.asarray(Wtt, f32)[0], np.asarray(Wto, f32)[0]], axis=0)
    ).reshape(1, 2, 128)
    ident = np.eye(128, dtype=f32)
    bias = np.ascontiguousarray(np.stack(
        [np.asarray(b, f32)[0:128], np.asarray(b, f32)[128:256],
         np.asarray(b, f32)[256:384], np.asarray(bt, f32),
         np.asarray(b, f32)[384:512], np.asarray(b1, f32),
         np.asarray(b2, f32), np.asarray(b3, f32),
         np.asarray(Wtt, f32)[0]], axis=1))                          # [128, 9]

    zero_bias = not (bias[:, 0:8].any())
    cfg = dict(DEFAULT_CFG, zero_bias=zero_bias)
    nc = _get_nc(("main", zero_bias), cfg)

    in_maps = []
    for core in range(N_CORES):
        rs = slice(core * R, (core + 1) * R)
        in_maps.append(dict(
            h=np.ascontiguousarray(hT[:, :, rs]),
            wk=wk, uh=uh, dw=dw, trow=trow, ident=ident, bias=bias,
            t=np.ascontiguousarray(tT[:, rs]).reshape(1, 3, R),
        ))

    global _LAST_IN_MAPS
    _LAST_IN_MAPS = in_maps
    res = run_bass_kernel_spmd(nc, in_maps, core_ids=list(range(N_CORES)))
    outs = [np.transpose(res.results[c]["out"], (2, 0, 1)) for c in range(N_CORES)]
    return np.ascontiguousarray(np.concatenate(outs, axis=0))



# revision 48
# speedup vs baseline: 1.1216x; 1.1216x over previous
"""Trainium2 Bass kernel for nn_Decode (3-step Time-LSTM decoder + dense stack).

Sharding: pure data parallel over batch across 8 NeuronCores (4096 rows each),
weights replicated. Device layout is feature-major (transposed): activations
are [feature_partition, batch_free] tiles, so all weights stay PE-stationary
and batch streams as the matmul moving operand (N=512 columns per chunk = one
PSUM bank at fp32).

Host-side prep (sharding/layout only):
  - slice context_state[:, 2, :] (the model reads only step 2)
  - fold the per-step attention vectors into Wx/Wxt:
        (h*aw_t) @ W == h @ (aw_t[:,None]*W); double the g columns so that
        tanh(g) can be evaluated as 2*sigmoid(2g)-1 on the sigmoid path
  - transpose h to [HID, B]; broadcast t across partitions (replication only)

All matmuls and elementwise tiles run bf16 (rel err ~2e-3 vs 2e-2 tolerance);
PSUM gate banks are fp32.  Per step t, PSUM: gs = [i|f|o] + tgg = [Tg|g']:
  gates = Wk_t.T @ h_last (+ Uh.T @ h_t)
  o-bank  += Wto_j * t_b        (GpSimd scalar_tensor_tensor RMW in PSUM)
  Tg-bank += sigma(Wtt_j t_b)   (GpSimd RMW of the ACT-computed s tile)
  ifoT[0:3] = sigmoid(gs);  ifoT[3:5] = sigmoid(tgg)      (two fused ACT ops)
  p1 = 2*(Tg*g') - Tg  (= Tg*tanh g);  c' = f*c + i*p1;  h' = o*tanh(c')
  out_t = relu-dense x3 (h')

Schedule structure (cost-model-driven; every engine's per-chunk work fits
inside the ~3.7us chunk cycle):
  - t=0 skips the f gate entirely (f*c0 == 0).
  - sigma(Wtt_j t_b) is batched 4 chunks per ACT instruction; t arrives
    pre-broadcast via the GpSimd DMA queue (no on-device broadcast).
  - tanh(c) and the h-multiply are emitted one chunk late so the ACT stream
    never head-of-line blocks on the DVE cell chain.
  - the dense stack is software-pipelined 3 chunks deep (l0 of chunk c-1,
    l1 of c-2, l2 of c-3 at chunk c); engine split: DVE cell chain +
    relu l1/l2, Pool PSUM injections + relu l0, ACT sigmoids/tanh only.
  - t=2's last chunk is split in half to shorten the serial drain.
"""
import sys

sys.path.insert(0, "/opt/trn_rl_repo")

import numpy as np
import concourse.bacc as bacc
import concourse.tile as tile
from concourse import mybir
from concourse.bass_utils import run_bass_kernel_spmd

N_CORES = 8
B = 32768
HID = 256
FEAT = 128
R = B // N_CORES        # batch rows per core
NB = 512                # batch columns per chunk (= one PSUM bank at fp32)
NCHUNK = R // NB
F32 = mybir.dt.float32
BF16 = mybir.dt.bfloat16
AF = mybir.ActivationFunctionType
ALU = mybir.AluOpType

DEFAULT_CFG = dict(
    zero_bias=True,   # fused sigmoids across gate banks (requires zero biases)
    split_last=True,  # halve the final two chunks of t=2 to shorten the drain
    # chunks per batched sigma(Wtt t) ACT instruction, per step: big batches
    # amortize instruction overhead while ACT still has slack (t=0), singles
    # avoid stretching the tight steady-state cycles (t>0)
    squads=(4, 1, 1),
)


def build_nc(cfg=None):
    cfg = {**DEFAULT_CFG, **(cfg or {})}
    zero_bias = cfg["zero_bias"]
    squads = cfg["squads"]

    nc = bacc.Bacc(target_bir_lowering=False)

    h_d = nc.dram_tensor("h", [2, 128, R], BF16, kind="ExternalInput")
    wk_d = nc.dram_tensor("wk", [2, 128, 3, 640], BF16, kind="ExternalInput")
    uh_d = nc.dram_tensor("uh", [128, 4, 128], BF16, kind="ExternalInput")
    dw_d = nc.dram_tensor("dw", [128, 3, 128], BF16, kind="ExternalInput")
    trow_d = nc.dram_tensor("trow", [1, 2, 128], BF16, kind="ExternalInput")
    ident_d = nc.dram_tensor("ident", [128, 128], BF16, kind="ExternalInput")
    bias_d = nc.dram_tensor("bias", [128, 10], F32, kind="ExternalInput")
    # t pre-broadcast across partitions on the host (replication only)
    t_d = nc.dram_tensor("t", [128, 3, R], BF16, kind="ExternalInput")
    out_d = nc.dram_tensor("out", [3, 128, R], F32, kind="ExternalOutput")

    with tile.TileContext(nc) as tc:
        with (
            tc.tile_pool(name="const", bufs=1) as const,
            tc.tile_pool(name="act", bufs=2) as act,
            tc.tile_pool(name="ps", bufs=1, space="PSUM") as ps,
        ):
            bias_sb = const.tile([128, 10], F32)
            nc.gpsimd.dma_start(out=bias_sb[:], in_=bias_d[:])
            trow_sb = const.tile([1, 2, 128], BF16)
            nc.gpsimd.dma_start(out=trow_sb[:], in_=trow_d[:])
            ident_sb = const.tile([128, 128], BF16)
            nc.gpsimd.dma_start(out=ident_sb[:], in_=ident_d[:])
            # warm the ACT table set (sigmoid/tanh) before data arrives
            warm = const.tile([1, 2], F32)
            nc.vector.memset(warm[:], 0.0)
            nc.scalar.activation(warm[:, 0:1], warm[:, 0:1], AF.Sigmoid)
            nc.scalar.activation(warm[:, 1:2], warm[:, 1:2], AF.Tanh)
            wk_sb = const.tile([128, 2, 3, 640], BF16)
            hsb = const.tile([128, 2, R], BF16)
            dw_sb = const.tile([128, 3, 128], BF16)
            uh_sb = const.tile([128, 4, 128], BF16)
            wk_r = wk_d.rearrange("a p t m -> p a t m")
            h_r = h_d.rearrange("a p n -> p a n")

            # recurrent state, updated in place (the write of step t happens
            # after all step-t readers of the same column range).  h feeds
            # matmuls (bf16); c accumulates across steps and the dense stack
            # amplifies its error against the small output scale, so the c
            # chain stays fp32.
            h_st = const.tile([128, R], BF16, name="hst")
            c_st = const.tile([128, R], F32, name="cst")

            def make_insts(t):
                full = [(slice(c * NB, (c + 1) * NB), NB, f"{c}") for c in range(NCHUNK)]
                if t == 2 and cfg.get("split_last", False):
                    # taper: last two chunks in halves to shorten the drain
                    out = full[:-2]
                    for c in (NCHUNK - 2, NCHUNK - 1):
                        h0 = slice(c * NB, c * NB + NB // 2)
                        h1 = slice(c * NB + NB // 2, (c + 1) * NB)
                        out += [(h0, NB // 2, f"{c}a"), (h1, NB // 2, f"{c}b")]
                    return out
                return full

            # ---- batched S chain: sigma(Wtt_j * t_b) ----
            # sgroups[g] = (t, lo, hi): one ACT sigmoid covers chunks
            # [lo, hi); scheduled 2 blocks before first use, t tile 2 earlier
            sgroups = []
            for t in range(3):
                w = squads[t] * NB
                for lo in range(0, R, w):
                    sgroups.append((t, lo, min(lo + w, R)))
            inst_sg = {}      # (t, key) -> (gidx, offset within group)
            need_block = {}   # gidx -> first global block using it
            gb_of = {}
            gb = 0
            for t in range(3):
                for col, nb, key in make_insts(t):
                    g = next(i for i, (tt_, lo, hi) in enumerate(sgroups)
                             if tt_ == t and lo <= col.start < hi)
                    inst_sg[(t, key)] = (g, col.start - sgroups[g][1])
                    need_block.setdefault(g, gb)
                    gb_of[(t, key)] = gb
                    gb += 1
            n_blocks = gb
            emit_at = {g: max(1, need_block[g] - 2) for g in need_block}
            load_at = {g: max(1, emit_at[g] - 2) for g in need_block}

            t_tiles = {}     # gidx -> broadcast t tile
            s_tiles = {}     # gidx -> sigma values

            def load_t(g):
                t, lo, hi = sgroups[g]
                tt = act.tile([128, hi - lo], BF16, tag="t_tile", bufs=4,
                              name=f"tt_{g}")
                nc.sync.dma_start(out=tt[:], in_=t_d[:, t, lo:hi])
                t_tiles[g] = tt

            def emit_schain(g):
                tt = t_tiles[g]
                s_sb = act.tile([128, tt.shape[-1]], BF16, tag="s_sb", bufs=3,
                                name=f"s_{g}")
                nc.scalar.activation(s_sb[:], tt[:], AF.Sigmoid,
                                     scale=bias_sb[:, 8:9])
                s_tiles[g] = s_sb

            # fill on the sync queue in need-by order: wk[t0] first (first
            # matmuls), then t group 0 (S chain + o-injection), then h chunks
            for m in range(5):
                nc.sync.dma_start(out=wk_sb[:, :, 0, m * 128:(m + 1) * 128],
                                  in_=wk_r[:, :, 0, m * 128:(m + 1) * 128])
            load_t(0)
            emit_schain(0)
            nc.sync.dma_start(out=hsb[:, :, 0:NB], in_=h_r[:, :, 0:NB])
            load_t(1)
            nc.sync.dma_start(out=hsb[:, :, NB:2 * NB], in_=h_r[:, :, NB:2 * NB])
            nc.sync.dma_start(out=dw_sb[:], in_=dw_d[:])
            for c in range(2, NCHUNK):
                col = slice(c * NB, (c + 1) * NB)
                nc.sync.dma_start(out=hsb[:, :, col], in_=h_r[:, :, col])
                if c == 2:
                    nc.sync.dma_start(out=uh_sb[:], in_=uh_d[:])
                    nc.sync.dma_start(out=wk_sb[:, :, 1, :], in_=wk_r[:, :, 1, :])
                if c == 4:
                    nc.sync.dma_start(out=wk_sb[:, :, 2, :], in_=wk_r[:, :, 2, :])
            sg_load = [2]
            sg_emit = [1]

            def emit_gates(t, inst, h_prev):
                """PE gate matmuls + GpSimd PSUM injections for one chunk.

                gs = [i|f|o] (t=0: [i|o]) with o emitted FIRST so its GpSimd
                Wto-injection hides behind the i/f matmuls; tgg = [Tg|g']
                with Tg first for the same reason."""
                col, nb, key = inst
                g, qoff = inst_sg[(t, key)]
                t_sl = t_tiles[g][:, qoff:qoff + nb]
                s_sl = s_tiles[g][:, qoff:qoff + nb]
                nbank = 3 if t > 0 else 2
                gs = ps.tile([128, nbank, nb], F32, tag="gs", name=f"gs_{key}_{t}")
                tgg = ps.tile([128, 2, nb], F32, tag="tgg", name=f"tgg_{key}_{t}")
                o_slot = nbank - 1

                def bank(tgt, m, uh_row, extra=None):
                    n_ex = (1 if (t > 0 and uh_row is not None) else 0) \
                        + (1 if extra is not None else 0)
                    for k in range(2):
                        nc.tensor.matmul(
                            tgt, wk_sb[:, k, t, m * 128:(m + 1) * 128],
                            hsb[:, k, col],
                            start=(k == 0), stop=(k == 1 and n_ex == 0),
                        )
                    if t > 0 and uh_row is not None:
                        n_ex -= 1
                        nc.tensor.matmul(
                            tgt, uh_sb[:, uh_row, :], h_prev[:, col],
                            start=False, stop=(n_ex == 0),
                        )
                    if extra is not None:
                        extra(tgt, True)

                # gs banks first ([o] then i, f) so the fused [i|f|o] sigmoid
                # is reachable after 10 matmuls; the Wto term accumulates
                # inside the o group (K=1 row of the broadcast t tile)
                def wto(tgt, last):
                    nc.tensor.matmul(tgt, trow_sb[:, 1, :], t_sl[0:1, :],
                                     start=False, stop=last)

                def ident(tgt, last):
                    nc.tensor.matmul(tgt, ident_sb[:], s_sl,
                                     start=False, stop=last)

                bank(gs[:, o_slot, :], 2, 2, extra=wto)
                bank(gs[:, 0, :], 0, 0)
                if t > 0:
                    bank(gs[:, 1, :], 1, 1)
                # tgg banks refill during the [i|f|o] sigmoid
                bank(tgg[:, 0, :], 3, None, extra=ident)
                bank(tgg[:, 1, :], 4, 3)
                return gs, tgg

            def emit_sigmas(t, gs, tgg, ifoT_dst):
                """fused sigmoids: [i|f|o] -> slots 0:3, [Tg|g'] -> slots 3:5
                (t=0: [i|o] -> 0:2, [Tg|g'] -> 2:4)."""
                nbank = 3 if t > 0 else 2
                if zero_bias:
                    nc.scalar.activation(ifoT_dst[:, 0:nbank, :], gs[:], AF.Sigmoid)
                    nc.scalar.activation(ifoT_dst[:, nbank:nbank + 2, :], tgg[:],
                                         AF.Sigmoid)
                else:
                    srcs = [gs[:, j, :] for j in range(nbank)] \
                        + [tgg[:, 0, :], tgg[:, 1, :]]
                    bidx = ([0, 1, 2] if t > 0 else [0, 2]) + [3, 4]
                    for j in range(nbank + 2):
                        nc.scalar.activation(ifoT_dst[:, j, :], srcs[j],
                                             AF.Sigmoid,
                                             bias=bias_sb[:, bidx[j]:bidx[j] + 1])

            # ---- software-pipelined dense stack ----
            # l0 of chunk c-2, l1 of c-3, l2 of c-4 are emitted at chunk c's
            # block head, so every dense dependency is >= 2 chunks old and
            # neither PE nor Pool ever head-of-line blocks on fresh results.
            dense_pend = []   # items: [t, col, nb, key, stage, cur_tile, idx]

            def advance_dense(bidx):
                for it in [x for x in dense_pend if x[4] == 2] \
                        + [x for x in dense_pend if x[4] == 1] \
                        + [x for x in dense_pend if x[4] == 0]:
                    t, col, nb, key, l, cur, idx = it
                    if l == 0 and idx > bidx - 2:
                        continue
                    dps = ps.tile([128, nb], F32, tag="dps", bufs=3,
                                  name=f"dps_{key}_{t}_{l}")
                    nc.tensor.matmul(
                        dps[:], dw_sb[:, l, :],
                        h_st[:, col] if l == 0 else cur[:],
                        start=True, stop=True,
                    )
                    odt = F32 if l == 2 else BF16
                    dsb = act.tile([128, nb], odt, tag=f"dsb{l}", bufs=3,
                                   name=f"d_{key}_{t}_{l}")
                    if not zero_bias:
                        nc.scalar.activation(
                            dsb[:], dps[:], AF.Relu, bias=bias_sb[:, 5 + l:6 + l]
                        )
                    else:
                        nc.vector.tensor_relu(dsb[:], dps[:])
                    it[4] += 1
                    it[5] = dsb
                    if l == 2:
                        nc.sync.dma_start(out=out_d[t, :, col], in_=dsb[:])
                dense_pend[:] = [x for x in dense_pend if x[4] < 3]

            # ---- lagged tanh(c) + h-multiply (one chunk late) ----
            tanh_pend = []    # items: (t, col, nb, key, o_slice)

            def flush_tanh():
                for t, col, nb, key, o_sl in tanh_pend:
                    tanh_c = act.tile([128, nb], BF16, tag="tanh_c",
                                      name=f"tc_{key}_{t}")
                    nc.scalar.activation(tanh_c[:], c_st[:, col], AF.Tanh)
                    nc.gpsimd.tensor_mul(h_st[:, col], o_sl, tanh_c[:])
                tanh_pend.clear()

            for t in range(3):
                for ii, inst in enumerate(make_insts(t)):
                    col, nb, key = inst
                    gb = gb_of[(t, key)]
                    while sg_load[0] < len(sgroups) and \
                            load_at[sg_load[0]] <= gb:
                        load_t(sg_load[0])
                        sg_load[0] += 1
                    gs, tgg = emit_gates(t, inst, h_st)
                    # slots: t>0 [i,f,o,Tg,g']; t=0 [i,o,Tg,g'].  fp32: bf16
                    # sigmoid outputs alone cost 6.7e-2 rel err (the 2g'-1
                    # reconstruction amplifies quantization by ~4x)
                    nslot = 5 if t > 0 else 4
                    ifoT = act.tile([128, nslot, nb], F32, tag="ifoT", bufs=3,
                                    name=f"ifoT_{key}_{t}")
                    emit_sigmas(t, gs, tgg, ifoT[:])
                    # ACT filler work while PE refills the gate banks:
                    # upcoming S chains + the lagged tanh(c)/h of chunk c-1
                    while sg_emit[0] < len(sgroups) and \
                            emit_at[sg_emit[0]] <= gb:
                        emit_schain(sg_emit[0])
                        sg_emit[0] += 1
                    flush_tanh()
                    i_g = ifoT[:, 0, :]
                    f_g = ifoT[:, 1, :] if t > 0 else None
                    o_g = ifoT[:, nslot - 3, :]
                    tg_g = ifoT[:, nslot - 2, :]
                    gp_g = ifoT[:, nslot - 1, :]

                    # ---- cell chain: p1 = Tg*tanh(g) = Tg*(2g'-1); the
                    # multiplies run on GpSimd (SBUF only), the rest on DVE
                    p0 = act.tile([128, nb], F32, tag="p0", name=f"p0_{key}_{t}")
                    nc.vector.tensor_scalar(
                        out=p0[:], in0=gp_g, scalar1=2.0, scalar2=1.0,
                        op0=ALU.mult, op1=ALU.subtract)
                    p1 = act.tile([128, nb], BF16, tag="p1", name=f"p1_{key}_{t}")
                    nc.gpsimd.tensor_mul(p1[:], tg_g, p0[:])
                    if t == 0:
                        nc.vector.tensor_mul(c_st[:, col], i_g, p1[:])
                    else:
                        p2 = act.tile([128, nb], F32, tag="p2", name=f"p2_{key}_{t}")
                        nc.vector.tensor_mul(p2[:], i_g, p1[:])
                        p3 = act.tile([128, nb], F32, tag="p3", name=f"p3_{key}_{t}")
                        nc.gpsimd.tensor_mul(p3[:], f_g, c_st[:, col])
                        nc.vector.tensor_add(c_st[:, col], p2[:], p3[:])

                    # dense stages of older chunks (PE slots after this
                    # chunk's gate matmuls), then queue this chunk
                    advance_dense(gb)
                    tanh_pend.append((t, col, nb, key, o_g))
                    dense_pend.append([t, col, nb, key, 0, None, gb])

            # drain: last tanh/h, then the dense pipeline
            flush_tanh()
            for _ in range(4):
                advance_dense(n_blocks + 4)

    nc.finalize()
    return nc


_NC_CACHE = {}


def _get_nc(key, cfg):
    if key not in _NC_CACHE:
        _NC_CACHE[key] = build_nc(cfg)
    return _NC_CACHE[key]


def _to_bf16(a):
    import ml_dtypes
    return np.asarray(a, np.float32).astype(ml_dtypes.bfloat16)


def kernel(context_state, input_t, aw1, aw2, aw3, Wx, Uh, b,
           Wxt, Wtt, bt, Wto, w1, b1, w2, b2, w3, b3):
    f32 = np.float32
    f64 = np.float64

    # ---- host-side prep / sharding ----
    h_last = np.asarray(context_state)[:, 2, :].astype(f32)          # [B, HID]
    hT = np.ascontiguousarray(h_last.T).reshape(2, 128, B)           # [2,128,B]
    tT = np.ascontiguousarray(np.asarray(input_t)[:, 3:, 0].T)       # [3, B]
    aw = np.concatenate(
        [np.asarray(aw1), np.asarray(aw2), np.asarray(aw3)], axis=1
    )[0].astype(f64)                                                 # [3, HID]

    Wx64, Wxt64 = np.asarray(Wx, f64), np.asarray(Wxt, f64)
    wk = np.empty((HID, 3, 640), f64)
    for t in range(3):
        wxf = aw[t][:, None] * Wx64                                  # [HID, 512]
        wtf = aw[t][:, None] * Wxt64                                 # [HID, 128]
        wk[:, t, 0:384] = wxf[:, 0:384]          # i, f, o
        wk[:, t, 384:512] = wtf                  # Tg
        wk[:, t, 512:640] = 2.0 * wxf[:, 384:512]  # g (doubled: tanh via 2s-1)
    wk = _to_bf16(wk.astype(f32)).reshape(2, 128, 3, 640)

    uh4 = np.asarray(Uh, f32).reshape(128, 4, 128).copy()
    uh4[:, 3, :] *= 2.0                          # g row doubled as well
    uh = _to_bf16(uh4)
    dw = _to_bf16(np.stack(
        [np.asarray(w1, f32), np.asarray(w2, f32), np.asarray(w3, f32)], axis=1))
    trow = _to_bf16(
        np.stack([np.asarray(Wtt, f32)[0], np.asarray(Wto, f32)[0]], axis=0)
    ).reshape(1, 2, 128)
    ident = _to_bf16(np.eye(128, dtype=f32))
    bias = np.ascontiguousarray(np.stack(
        [np.asarray(b, f32)[0:128], np.asarray(b, f32)[128:256],
         np.asarray(b, f32)[256:384], np.asarray(bt, f32),
         np.asarray(b, f32)[384:512], np.asarray(b1, f32),
         np.asarray(b2, f32), np.asarray(b3, f32),
         np.asarray(Wtt, f32)[0], np.asarray(Wto, f32)[0]], axis=1))  # [128,10]

    zero_bias = not (bias[:, 0:8].any())
    cfg = dict(DEFAULT_CFG, zero_bias=zero_bias)
    nc = _get_nc(("main", zero_bias), cfg)

    hT16 = _to_bf16(hT)
    tT16 = _to_bf16(tT)
    in_maps = []
    for core in range(N_CORES):
        rs = slice(core * R, (core + 1) * R)
        t_core = np.ascontiguousarray(
            np.broadcast_to(tT16[None, :, rs], (128, 3, R)))
        in_maps.append(dict(
            h=np.ascontiguousarray(hT16[:, :, rs]),
            wk=wk, uh=uh, dw=dw, trow=trow, ident=ident, bias=bias,
            t=t_core,
        ))

    global _LAST_IN_MAPS
    _LAST_IN_MAPS = in_maps
    res = run_bass_kernel_spmd(nc, in_maps, core_ids=list(range(N_CORES)))
    outs = [np.transpose(res.results[c]["out"], (2, 0, 1)) for c in range(N_CORES)]
    return np.ascontiguousarray(np.concatenate(outs, axis=0))
